# revision 16
# baseline (speedup 1.0000x reference)
"""Trainium2 Bass kernel for the IWE (image-warped-events) problem, v3.

Full inputs in, full outputs out. Data-parallel over (batch, half) across 8
NeuronCores (core 2b+h gets half h of batch b); host sums the two partial
IWEs per batch.

v3 vs v2 (baseline): the per-core pipeline is restructured around the
measured bottleneck, the GPSIMD flow gather (~114us per superchunk, 16 of
them = 1.82ms envelope). All other engines are slimmed to fit under it:
  - polarity handled by a +/-1 sign trick with TWO PSUM accumulators
    (acc_tot, acc_sgn) instead of a 288-wide polarity-folded x grid:
    x one-hot window shrinks to 132 cols; pos = (tot+sgn)/2, neg=(tot-sgn)/2
  - warped x clamped to [-1, 128.5] so 2 spill cols on each side of the
    132-wide window absorb all OOB corners (matching the reference's OOB
    drop); y window is exactly 128 rows and drops OOB naturally
  - hats built in 3 ops each: T = iota - w (DVE TT), A = |T|-1 (DVE TS
    abs_max+sub fused), H = -min(A,0) (TS min+mult fused) -> positive hat;
    X-side abs/relu run on ACT (scalar engine) to keep DVE under the
    gather envelope
  - gather extraction: sel16 mask + 4-level strided TT add tree (replaces
    the 1x-rate tensor_reduce)
  - pad events use x=-500: clamped to the spill col -> zero contribution
"""
import numpy as np
import ml_dtypes

H, W = 128, 128
NCORES = 8
CHUNK = 500                    # kept for test.py's cache-key computation
E_REAL = 500000                # events per core (N/2)
NCOLS = 3968
E_PAD = 128 * NCOLS            # 507904
NSC = 16                       # superchunks
SC = NCOLS // NSC              # 248 event-columns per superchunk
NI = 16 * SC                   # gather idxs per Q7 core per superchunk
W2 = 132                       # x window: col c <-> x = c-1; real grid cols 1..128
NB = 8                         # blocks per hat group

_COMPILED = {}


def _build(nchunks, use_hw_loop=True, unroll=2, passes=1, ablate=None,
           n_dve_x=0):
    import concourse.bass as bass
    import concourse.bacc as bacc
    import concourse.mybir as mybir
    from concourse.tile import TileContext

    fp32 = mybir.dt.float32
    bf16 = mybir.dt.bfloat16
    int16 = mybir.dt.int16
    int32 = mybir.dt.int32
    Alu = mybir.AluOpType
    Act = mybir.ActivationFunctionType

    nc = bacc.Bacc("TRN2", target_bir_lowering=False, debug=False,
                   num_devices=NCORES)

    ev = nc.dram_tensor("ev", [E_PAD, 4], fp32, kind="ExternalInput").ap()
    ftab = nc.dram_tensor("ftab", [2 * H * W], bf16, kind="ExternalInput").ap()
    flow = nc.dram_tensor("flow", [2, H, W], fp32, kind="ExternalInput").ap()
    emask = nc.dram_tensor("emask", [H, W], fp32, kind="ExternalInput").ap()
    selin = nc.dram_tensor("sel16", [128, 32], bf16, kind="ExternalInput").ap()
    out = nc.dram_tensor("out", [4, H, W], fp32, kind="ExternalOutput").ap()

    ev_v = ev.rearrange("(p s c) f -> p s (c f)", p=128, s=NSC, c=SC)

    with TileContext(nc) as tc:
        with tc.tile_pool(name="const", bufs=1) as cpool, \
             tc.tile_pool(name="gpf", bufs=3) as gpool, \
             tc.tile_pool(name="work", bufs=2) as wpool, \
             tc.tile_pool(name="hats", bufs=2) as hpool, \
             tc.tile_pool(name="ppool", bufs=1, space="PSUM") as ppool:

            # ---------------- constants ----------------
            iotai = cpool.tile([128, 128], int32)
            nc.gpsimd.iota(iotai[:], pattern=[[1, 128]], base=0,
                           channel_multiplier=0)
            # y basis: 0..127
            iota = cpool.tile([128, 128], bf16)
            nc.vector.tensor_copy(out=iota[:], in_=iotai[:])
            # x basis: col c <-> x = c-1, values -1..130
            iotaf = cpool.tile([128, 128], fp32)
            nc.vector.tensor_copy(out=iotaf[:], in_=iotai[:])
            iotax = cpool.tile([128, W2], bf16)
            nc.vector.memset(iotax[:], 0.0)
            nc.vector.tensor_scalar(out=iotax[:, 0:128], in0=iotaf[:],
                                    scalar1=-1.0, scalar2=None, op0=Alu.add)
            nc.vector.memset(iotax[:, 128:129], 127.0)
            nc.vector.memset(iotax[:, 129:130], 128.0)
            nc.vector.memset(iotax[:, 130:131], 129.0)
            nc.vector.memset(iotax[:, 131:132], 130.0)
            c128 = cpool.tile([128, 1], fp32)
            nc.vector.memset(c128[:], 128.0)
            cinv = cpool.tile([128, 1], fp32)
            nc.vector.memset(cinv[:], 1.0 / (1.0 + 1e-9))

            # sel16[q, 2k+t] = (k == q%16): per-partition slot mask used to
            # extract each event's value from the core-replicated gather out
            sel16 = cpool.tile([128, 32], bf16)
            nc.sync.dma_start(out=sel16[:], in_=selin)

            table = cpool.tile([128, 2 * H * W], bf16)
            nc.sync.dma_start(
                out=table[:],
                in_=ftab.unsqueeze(0).broadcast_to([128, 2 * H * W]))

            flow32 = cpool.tile([128, 256], fp32)
            nc.sync.dma_start(out=flow32[:, 0:128], in_=flow[0])
            nc.sync.dma_start(out=flow32[:, 128:256], in_=flow[1])
            maskt = cpool.tile([128, 128], fp32)
            nc.sync.dma_start(out=maskt[:], in_=emask[:, :])

            acc_tot = ppool.tile([128, W2], fp32, tag="acc_tot")
            nc.vector.memset(acc_tot[:], 0.0)
            acc_sgn = ppool.tile([128, W2], fp32, tag="acc_sgn")
            nc.vector.memset(acc_sgn[:], 0.0)

            # ---------------- event pipeline ----------------
            # software-pipelined: prefetch(s) = event DMA + gather idx build +
            # GPSIMD gather issue; emitted one superchunk AHEAD of process(s-1)
            # so the gathers run back-to-back on GPSIMD instead of serializing
            # behind the previous superchunk's DVE hat work (the idx build is
            # tiny but used to sit after 31 groups of hats in DVE queue order)
            pf = {}

            def prefetch(s):
                evt = gpool.tile([128, SC * 4], fp32, tag="evt")
                nc.sync.dma_start(out=evt[:], in_=ev_v[:, bass.ds(s, 1), :])
                ev3 = evt[:].rearrange("p (c f) -> p c f", f=4)

                idxf = gpool.tile([128, SC], fp32, tag="idxf")
                nc.vector.scalar_tensor_tensor(
                    out=idxf[:], in0=ev3[:, :, 1], scalar=c128[:],
                    in1=ev3[:, :, 2], op0=Alu.mult, op1=Alu.add)
                idxi = gpool.tile([128, SC], int16, tag="idxi")
                nc.vector.tensor_copy(out=idxi[:], in_=idxf[:])

                gout = None
                if ablate != "nogather":
                    gout = gpool.tile([128, NI * 2], bf16, tag="gout")
                    nc.gpsimd.ap_gather(
                        gout[:], table[:], idxi[:],
                        channels=128, num_elems=H * W, d=2, num_idxs=NI)
                pf[s] = (evt, gout)

            def body(s):
                evt, gout = pf.pop(s)
                ev3 = evt[:].rearrange("p (c f) -> p c f", f=4)
                tsv = ev3[:, :, 0]
                yv = ev3[:, :, 1]
                xv = ev3[:, :, 2]
                pv = ev3[:, :, 3]

                fyfx = wpool.tile([128, SC * 2], fp32, tag="fyfx")
                f3 = fyfx[:].rearrange("p (j two) -> p j two", two=2)
                if ablate != "nogather":
                    # each partition keeps only its own slot (k == q%16) of
                    # the 16-replicated gather output, then sums the 16 slots
                    # with a strided TT add tree (bf16 2x mode; the 1x-rate
                    # tensor_reduce is ~2x slower)
                    nc.vector.tensor_tensor(
                        out=gout[:].rearrange("p (j kt) -> p j kt", kt=32),
                        in0=gout[:].rearrange("p (j kt) -> p j kt", kt=32),
                        in1=sel16[:].unsqueeze(1).broadcast_to([128, SC, 32]),
                        op=Alu.mult)
                    g4 = gout[:].rearrange("p (j k two) -> p j k two", k=16,
                                           two=2)
                    nc.vector.tensor_tensor(
                        out=g4[:, :, 0:8, :], in0=g4[:, :, 0:8, :],
                        in1=g4[:, :, 8:16, :], op=Alu.add)
                    nc.vector.tensor_tensor(
                        out=g4[:, :, 0:4, :], in0=g4[:, :, 0:4, :],
                        in1=g4[:, :, 4:8, :], op=Alu.add)
                    nc.vector.tensor_tensor(
                        out=g4[:, :, 0:2, :], in0=g4[:, :, 0:2, :],
                        in1=g4[:, :, 2:4, :], op=Alu.add)
                    nc.vector.tensor_tensor(
                        out=f3, in0=g4[:, :, 0, :],
                        in1=g4[:, :, 1, :], op=Alu.add)
                else:
                    nc.vector.memset(fyfx[:], 0.25)
                fy = f3[:, :, 0]
                fx = f3[:, :, 1]

                u = wpool.tile([128, SC], fp32, tag="u")
                nc.vector.tensor_scalar(out=u[:], in0=tsv, scalar1=-1.0,
                                        scalar2=1.0, op0=Alu.mult, op1=Alu.add)
                t1 = wpool.tile([128, SC], fp32, tag="t1")
                nc.vector.tensor_tensor(out=t1[:], in0=u[:], in1=fy, op=Alu.mult)
                wy = wpool.tile([128, SC], fp32, tag="wy")
                nc.vector.tensor_tensor(out=wy[:], in0=t1[:], in1=yv, op=Alu.add)
                t2 = wpool.tile([128, SC], fp32, tag="t2")
                nc.vector.tensor_tensor(out=t2[:], in0=u[:], in1=fx, op=Alu.mult)
                wx0 = wpool.tile([128, SC], fp32, tag="wx0")
                nc.vector.tensor_tensor(out=wx0[:], in0=t2[:], in1=xv, op=Alu.add)
                # clamp into [-1, 128.5]: far-left junk (incl. x=-500 pads)
                # lands on spill col 0 with zero real-grid weight; far-right
                # lands on spill cols 129/130
                wx = wpool.tile([128, SC], fp32, tag="wx")
                nc.vector.tensor_scalar(out=wx[:], in0=wx0[:], scalar1=-1.0,
                                        scalar2=128.5, op0=Alu.max, op1=Alu.min)
                # sign: +1 for p=1 (pos grid), -1 for p=0
                sgn = wpool.tile([128, SC], bf16, tag="sgn")
                nc.vector.tensor_scalar(out=sgn[:], in0=pv, scalar1=2.0,
                                        scalar2=-1.0, op0=Alu.mult, op1=Alu.add)

                if ablate == "nohats":
                    return
                for g in range(SC // NB):
                    TY = hpool.tile([128, NB * 128], bf16, tag="TY")
                    AY = hpool.tile([128, NB * 128], bf16, tag="AY")
                    HY = hpool.tile([128, NB * 128], bf16, tag="HY")
                    TX = hpool.tile([128, NB * W2], bf16, tag="TX")
                    AX = hpool.tile([128, NB * W2], bf16, tag="AX")
                    HX = hpool.tile([128, NB * W2], bf16, tag="HX")
                    HXS = hpool.tile([128, NB * W2], bf16, tag="HXS")
                    gs = slice(g * NB, (g + 1) * NB)
                    nc.vector.tensor_tensor(
                        out=TY[:].rearrange("p (b f) -> p b f", f=128),
                        in0=iota[:].unsqueeze(1).broadcast_to([128, NB, 128]),
                        in1=wy[:, gs].unsqueeze(2).broadcast_to([128, NB, 128]),
                        op=Alu.subtract)
                    nc.vector.tensor_tensor(
                        out=TX[:].rearrange("p (b f) -> p b f", f=W2),
                        in0=iotax[:].unsqueeze(1).broadcast_to([128, NB, W2]),
                        in1=wx[:, gs].unsqueeze(2).broadcast_to([128, NB, W2]),
                        op=Alu.subtract)
                    # negated hat: HY = min(|TY|,1) - 1 in [-1,0]; |TY| built
                    # on ACT (Abs) for even groups, DVE (neg+max) for odd, to
                    # balance the two engines under the gather envelope
                    if True:
                        nc.scalar.activation(out=AY[:], in_=TY[:], func=Act.Abs)
                    else:
                        TNY = hpool.tile([128, NB * 128], bf16, tag="TNY")
                        nc.vector.tensor_scalar_mul(out=TNY[:], in0=TY[:],
                                                    scalar1=-1.0)
                        nc.vector.tensor_tensor(out=AY[:], in0=TY[:],
                                                in1=TNY[:], op=Alu.max)
                    nc.vector.tensor_scalar(out=HY[:], in0=AY[:], scalar1=1.0,
                                            scalar2=1.0, op0=Alu.min,
                                            op1=Alu.subtract)
                    # positive x hat: HX = relu(1 - |TX|) on ACT
                    nc.scalar.activation(out=AX[:], in_=TX[:], func=Act.Abs)
                    nc.scalar.activation(out=HX[:], in_=AX[:],
                                         func=Act.Relu, bias=1.0,
                                         scale=-1.0)
                    # sign-folded copy of the x one-hot for the +/- accumulator
                    nc.vector.tensor_tensor(
                        out=HXS[:].rearrange("p (b f) -> p b f", f=W2),
                        in0=HX[:].rearrange("p (b f) -> p b f", f=W2),
                        in1=sgn[:, gs].unsqueeze(2).broadcast_to([128, NB, W2]),
                        op=Alu.mult)
                    if ablate == "nomm":
                        continue
                    for b in range(NB):
                        nc.tensor.matmul(
                            out=acc_tot[:],
                            lhsT=HY[:, b * 128:(b + 1) * 128],
                            rhs=HX[:, b * W2:(b + 1) * W2],
                            start=False, stop=False)
                        nc.tensor.matmul(
                            out=acc_sgn[:],
                            lhsT=HY[:, b * 128:(b + 1) * 128],
                            rhs=HXS[:, b * W2:(b + 1) * W2],
                            start=False, stop=False)

            # event pipeline, fully unrolled over superchunks (no inner HW
            # loop: its per-back-edge all-engine barrier would serialize the
            # GPSIMD gather against the DVE/ACT hat work); passes>1 (timing
            # variant) wraps it in a static outer HW loop
            def pipeline():
                prefetch(0)
                prefetch(1)
                for s in range(NSC):
                    if s + 2 < NSC:
                        prefetch(s + 2)
                    body(s)

            if passes == 1:
                pipeline()
            else:
                with tc.For_i(0, passes):
                    pipeline()

            # ---------------- finalize ----------------
            # grids live in cols 1..129 of the accumulators (x = col-1)
            tot = cpool.tile([128, W2], fp32)
            nc.vector.tensor_copy(out=tot[:], in_=acc_tot[:])
            sg = cpool.tile([128, W2], fp32)
            nc.vector.tensor_copy(out=sg[:], in_=acc_sgn[:])
            res = cpool.tile([128, 512], fp32)
            # acc holds -(hatY*hatX) sums (HY negated, HX positive):
            # pos = -(tot+sgn)/2, neg = -(tot-sgn)/2
            nc.vector.tensor_tensor(out=res[:, 0:128], in0=tot[:, 1:129],
                                    in1=sg[:, 1:129], op=Alu.add)
            nc.vector.tensor_scalar_mul(out=res[:, 0:128], in0=res[:, 0:128],
                                        scalar1=-0.5)
            nc.vector.tensor_tensor(out=res[:, 128:256], in0=tot[:, 1:129],
                                    in1=sg[:, 1:129], op=Alu.subtract)
            nc.vector.tensor_scalar_mul(out=res[:, 128:256],
                                        in0=res[:, 128:256], scalar1=-0.5)
            nc.vector.scalar_tensor_tensor(
                out=res[:, 256:384], in0=flow32[:, 0:128], scalar=cinv[:],
                in1=maskt[:], op0=Alu.mult, op1=Alu.mult)
            nc.vector.scalar_tensor_tensor(
                out=res[:, 384:512], in0=flow32[:, 128:256], scalar=cinv[:],
                in1=maskt[:], op0=Alu.mult, op1=Alu.mult)
            for ch in range(4):
                nc.sync.dma_start(out=out[ch],
                                  in_=res[:, ch * 128:(ch + 1) * 128])

    nc.compile()
    return nc


def _run(nc, flow, event_list, pol_mask, event_mask):
    """flow [B,2,H,W], event_list [B,N,4], pol [B,N,2], emask [B,1,H,W]."""
    from concourse.bass_utils import run_bass_kernel_spmd

    Bb, Nn = event_list.shape[0], event_list.shape[1]
    half = Nn // 2
    pad = np.zeros((E_PAD - E_REAL, 4), np.float32)
    pad[:, 2] = -500.0           # x=-500: clamped to spill col, contributes 0
    sel16 = np.zeros((128, 32), ml_dtypes.bfloat16)
    for q in range(128):
        sel16[q, 2 * (q % 16)] = 1.0
        sel16[q, 2 * (q % 16) + 1] = 1.0
    ftabs = []
    for b in range(Bb):
        t = np.empty(2 * H * W, ml_dtypes.bfloat16)
        t[0::2] = flow[b, 1].ravel().astype(ml_dtypes.bfloat16)   # fy
        t[1::2] = flow[b, 0].ravel().astype(ml_dtypes.bfloat16)   # fx
        ftabs.append(t)
    in_maps = []
    for c in range(NCORES):
        b, h = c // 2, c % 2
        sl = slice(h * half, (h + 1) * half)
        evc = np.concatenate(
            [np.ascontiguousarray(event_list[b, sl, :], np.float32), pad])
        in_maps.append({
            "ev": evc,
            "ftab": ftabs[b],
            "flow": np.ascontiguousarray(flow[b], np.float32),
            "emask": np.ascontiguousarray(event_mask[b, 0], np.float32),
            "sel16": sel16,
        })
    res = run_bass_kernel_spmd(nc, in_maps, list(range(NCORES)))
    outp = np.zeros((Bb, 4, H, W), np.float32)
    for c in range(NCORES):
        b = c // 2
        r = res.results[c]["out"]
        outp[b, 0:2] += r[0:2]
        if c % 2 == 0:
            outp[b, 2:4] = r[2:4]
    return outp


def kernel(flow, event_list, pol_mask, event_mask):
    flow = np.asarray(flow, np.float32)
    event_list = np.asarray(event_list, np.float32)
    pol_mask = np.asarray(pol_mask, np.float32)
    event_mask = np.asarray(event_mask, np.float32)
    nchunks = event_list.shape[0] * event_list.shape[1] // NCORES // CHUNK
    key = ("nc", nchunks)
    if key not in _COMPILED:
        _COMPILED[key] = _build(nchunks)
    return _run(_COMPILED[key], flow, event_list, pol_mask, event_mask)
